# revision 28
# baseline (speedup 1.0000x reference)
"""TRN2 Bass kernel: masked multi-head attention block (B=4, S=2048, C=768, H=12).

Sharding: 8 cores = 4 batches x 2 head-groups (6 heads each).  Each core runs a
flash-attention-style Bass/Tile kernel over its (batch, head-group) shard:

  qT/kT: [384, S] feature-major bf16 projections from xT (q pre-scaled by hd^-0.5)
  v:     [S, 6*65] bf16 natural layout, a ones column appended per head
  scoresT[k, q] = k . q (contract hd=64, head pairs row-packed on the PE array)
  pT = exp(scoresT) on the scalar engine (psum f32 -> sbuf bf16)
  pT *= mask01T on DVE (bf16 2x mode); mask01 = 1 - mask, so masked keys get
  exactly 0 (reference adds -1e5 pre-softmax: exp(s-1e5) ~ 0)
  avT[65, 512] accumulated over key chunks; row 64 = softmax denominator
  attn_outT = avT[0:64] * recip(denominator)  (partition-broadcast on gpsimd)
  y = attn_outT.T @ w_projT slice  (row-parallel output projection)

Pipelining: one PSUM ring ([128,2,512] slots) is shared by the qkv-projection
units, the score groups, and the output-projection units, so all three phases
interleave in the PE stream: attention on (qb0, pair0) starts as soon as the
m=0 q/k slices and v are projected, the remaining q/k slices ride along with
qb0's score groups, and each query block's output projection + DMA rides along
with the next query block.  AV matmuls lag the score matmuls by 2 groups so
the PE never waits on the exp/mask chain (keeps the HAM p-state warm).
Host-side: transposes/slices the weights per core, builds the bf16 {0,1} mask,
sums the two per-batch partials, and adds b_proj.
"""

from contextlib import ExitStack

import numpy as np

import concourse.tile as tile
from concourse import bacc, mybir
from concourse.bass_utils import run_bass_kernel_spmd

F32 = mybir.dt.float32
BF16 = mybir.dt.bfloat16

B, S, C, H = 4, 2048, 768, 12
HD = 64
H_PER_CORE = 6
D_CORE = H_PER_CORE * HD  # 384
QBLK = 512
GRP = 2
N_CORES = 8
AV_LAG = 2


def _build_kernel():
    nc = bacc.Bacc(
        trn_type="TRN2", target_bir_lowering=False, debug=False, num_devices=N_CORES
    )
    KC = S // 128  # 16 key chunks
    QB = S // QBLK  # 4 query blocks
    ST = S // 128
    groups = []
    for half in range(KC // 8):
        base = half * 8
        for g0 in (0, 2, 4, 6):
            groups.append(base + g0)

    # all inputs pre-arranged host-side into the exact SBUF layout so every
    # DMA is 128 big contiguous per-partition descriptors
    xw = nc.dram_tensor(
        "xw", [128, 6, S + 3 * D_CORE], BF16, kind="ExternalInput"
    ).ap()
    wproj = nc.dram_tensor("wproj", [128, 3, C], BF16, kind="ExternalInput").ap()
    vones = nc.dram_tensor("vones", [128, S // 128 * H_PER_CORE], BF16, kind="ExternalInput").ap()
    maskT = nc.dram_tensor(
        "maskT", [S // QBLK, KC // 8, 128, 8, QBLK], BF16, kind="ExternalInput"
    ).ap()
    y = nc.dram_tensor("y", [S, C], F32, kind="ExternalOutput").ap()

    with tile.TileContext(nc) as tc, ExitStack() as ctx:
        consts = ctx.enter_context(tc.tile_pool(name="consts", bufs=1))
        qkv_pool = ctx.enter_context(tc.tile_pool(name="qkv", bufs=1))
        mpool = ctx.enter_context(tc.tile_pool(name="mask", bufs=4))
        xpool = ctx.enter_context(tc.tile_pool(name="x1", bufs=1))
        ppool = ctx.enter_context(tc.tile_pool(name="pT", bufs=8))
        dpool = ctx.enter_context(tc.tile_pool(name="div", bufs=2))
        bpool = ctx.enter_context(tc.tile_pool(name="bcast", bufs=2))
        ypool = ctx.enter_context(tc.tile_pool(name="y", bufs=3))
        ps = ctx.enter_context(tc.tile_pool(name="ps", bufs=3, space="PSUM"))
        ps_av = ctx.enter_context(tc.tile_pool(name="ps_av", bufs=2, space="PSUM"))

        # ---- input DMAs: x+weights fused into one tile, 3 big transfers
        xw_sb = xpool.tile([128, 6, S + 3 * D_CORE], BF16)
        xT_sb = xw_sb[:, :, :S]
        wk_sb = xw_sb[:, :, S : S + D_CORE]
        wq_sb = xw_sb[:, :, S + D_CORE : S + 2 * D_CORE]
        wv_sb = xw_sb[:, :, S + 2 * D_CORE :]
        nc.sync.dma_start(
            xw_sb[:, :, S : S + D_CORE], xw[:, :, S : S + D_CORE]
        )
        nc.sync.dma_start(xw_sb[:, :, : 2 * QBLK], xw[:, :, : 2 * QBLK])
        nc.sync.dma_start(xw_sb[:, :, S + D_CORE :], xw[:, :, S + D_CORE :])
        nc.sync.dma_start(xw_sb[:, :, 2 * QBLK : S], xw[:, :, 2 * QBLK : S])
        wproj_sb = consts.tile([128, 3, C], BF16)
        nc.sync.dma_start(wproj_sb[:], wproj[:])

        qT_sb = qkv_pool.tile([128, 3, S], BF16)
        kT_sb = qkv_pool.tile([128, 3, S], BF16)
        vaug_sb = qkv_pool.tile([128, ST, H_PER_CORE * (HD + 1)], BF16)
        attn_sb = qkv_pool.tile([128, 3, S], BF16)

        vaug_ones = vaug_sb.rearrange("p st (h u) -> p st h u", u=HD + 1)[:, :, :, HD]
        nc.sync.dma_start(
            vaug_ones, vones.rearrange("p (st h) -> p st h", h=H_PER_CORE)
        )

        mask_cache = {}

        def load_mask(qb_i):
            halves = []
            for half in range(KC // 8):
                mh = mpool.tile([128, 8, QBLK], BF16, tag="mask", name="mask_h")
                nc.sync.dma_start(mh[:], maskT[qb_i, half])
                halves.append(mh)
            return halves

        mask_cache[0] = load_mask(0)

        # ---- unit emitters, all drawing PSUM from the shared ring ----
        qk_cp = [0]

        def emit_qk_unit(w_sb, dst, m, nb0, dve_copy=False):
            """Project two q-blocks of q or k for weight column block m."""
            u = ps.tile([128, GRP, QBLK], F32, tag="sc", name="psqk")
            for j in range(2):
                for k in range(6):
                    nc.tensor.matmul(
                        u[:, j, :],
                        w_sb[:, k, m * 128 : (m + 1) * 128],
                        xT_sb[:, k, (nb0 + j) * QBLK : (nb0 + j + 1) * QBLK],
                        start=(k == 0),
                        stop=(k == 5),
                    )
            dst_ap = dst[:, m, nb0 * QBLK : (nb0 + 2) * QBLK].rearrange(
                "p (j q) -> p j q", j=2
            )
            if dve_copy or qk_cp[0] % 2 == 0:
                nc.vector.tensor_copy(dst_ap, u[:])
            else:
                nc.scalar.copy(dst_ap, u[:])
            qk_cp[0] += 1

        def emit_v_unit(st):
            u = ps.tile([128, GRP, QBLK], F32, tag="sc", name="psv")
            pv = u[:, 0, :D_CORE]
            for k in range(6):
                nc.tensor.matmul(
                    pv,
                    xT_sb[:, k, st * 128 : (st + 1) * 128],
                    wv_sb[:, k, :],
                    start=(k == 0),
                    stop=(k == 5),
                )
            for h in range(H_PER_CORE):
                dst = vaug_sb[:, st, h * (HD + 1) : h * (HD + 1) + HD]
                src = u[:, 0, h * HD : (h + 1) * HD]
                if h % 2 == 0:
                    nc.vector.tensor_copy(dst, src)
                else:
                    nc.scalar.copy(dst, src)

        y_r = y.rearrange("(st p) o -> st p o", p=128)

        def emit_proj_unit(st):
            u = ps.tile([128, GRP, QBLK], F32, tag="sc", name="psy")
            y_sb = ypool.tile([128, C], F32, tag="ysb", name="y_sb")
            for nb2 in range(2):
                for k3 in range(3):
                    nc.tensor.matmul(
                        u[:, nb2, :384],
                        attn_sb[:, k3, st * 128 : (st + 1) * 128],
                        wproj_sb[:, k3, nb2 * 384 : (nb2 + 1) * 384],
                        start=(k3 == 0),
                        stop=(k3 == 2),
                    )
            nc.vector.tensor_copy(
                y_sb.rearrange("p (j o) -> p j o", j=2), u[:, :, :384]
            )
            nc.sync.dma_start(y_r[st], y_sb[:])

        # ---- attention machinery ----
        pending = []

        def issue_av(ent):
            qb, hp, g0, av, pT_pair = ent
            for i, h in ((0, 2 * hp), (1, 2 * hp + 1)):
                for c in range(GRP):
                    kc = g0 + c
                    nc.tensor.matmul(
                        av[i][:],
                        vaug_sb[:, kc, h * (HD + 1) : (h + 1) * (HD + 1)],
                        pT_pair[i][:, c, :],
                        start=(kc == 0),
                        stop=(kc == KC - 1),
                    )
            if g0 + GRP == KC:
                finalize_pair(qb, hp, av)

        def finalize_pair(qb, hp, av):
            av_all = dpool.tile(
                [HD + 1, 2, QBLK], F32, tag="av_all", name="av_all", bufs=2
            )
            for i in (0, 1):
                nc.vector.tensor_copy(av_all[:, i, :], av[i][:])
            dstack = dpool.tile([2, QBLK], F32, tag="dstack", name="dstack", bufs=2)
            recip = dpool.tile([2, QBLK], F32, tag="recip", name="recip", bufs=2)
            nc.gpsimd.dma_start(dstack[:], av_all[HD : HD + 1, :, :])
            nc.vector.reciprocal_approx_fast(recip[:], dstack[:])
            r2 = bpool.tile([1, 2, QBLK], F32, tag="r2", name="r2", bufs=2)
            nc.gpsimd.dma_start(r2[:], recip[:])
            tmp = bpool.tile([HD, QBLK], BF16, tag="tmp", name="tmp", bufs=2)
            bcs = []
            for i in (0, 1):
                bc = bpool.tile([HD, QBLK], F32, tag="bc", name="bc", bufs=6)
                nc.gpsimd.partition_broadcast(bc[:], r2[:, i, :])
                bcs.append(bc)
            qsl = slice(qb * QBLK, (qb + 1) * QBLK)
            nc.vector.tensor_mul(
                attn_sb[:HD, hp, qsl], av_all[:HD, 0, :], bcs[0][:]
            )
            nc.vector.tensor_mul(tmp[:], av_all[:HD, 1, :], bcs[1][:])
            nc.sync.dma_start(attn_sb[HD:128, hp, qsl], tmp[:])

        def emit_group(qb, hp, g0, mask_halves, av):
            hA, hB = 2 * hp, 2 * hp + 1
            mh = mask_halves[g0 // 8]
            moff = g0 % 8
            sc = [
                ps.tile([128, GRP, QBLK], F32, tag="sc", name="scA")[:],
                ps.tile([128, GRP, QBLK], F32, tag="sc", name="scB")[:],
            ]
            for i, h in ((0, hA), (1, hB)):
                row0 = (h % 2) * HD
                for c in range(GRP):
                    kc = g0 + c
                    nc.tensor.matmul(
                        sc[i][:, c, :],
                        kT_sb[row0 : row0 + HD, h // 2, kc * 128 : (kc + 1) * 128],
                        qT_sb[row0 : row0 + HD, h // 2, qb * QBLK : (qb + 1) * QBLK],
                        start=True,
                        stop=True,
                        tile_position=(row0, 0),
                    )
            pT_views = []
            for i in (0, 1):
                pT = ppool.tile(
                    [128, GRP, QBLK], BF16, tag="pT2", name="pT2", bufs=4
                )
                nc.scalar.activation(
                    pT[:], sc[i], mybir.ActivationFunctionType.Exp
                )
                pT_views.append(pT[:])
            pT_pair = []
            for i in (0, 1):
                pTm = ppool.tile([128, GRP, QBLK], BF16, tag="pTm", name="pTm", bufs=10)
                nc.vector.tensor_mul(pTm[:], pT_views[i], mh[:, moff : moff + GRP, :])
                pT_pair.append(pTm)
            pending.append((qb, hp, g0, av, pT_pair))
            if len(pending) > AV_LAG:
                issue_av(pending.pop(0))

        # ---- schedule ----
        # startup: project m=0 slices of k and q, then all of v
        for nb0 in (0, 2):
            emit_qk_unit(wk_sb, kT_sb, 0, nb0)
        for nb0 in (0, 2):
            emit_qk_unit(wq_sb, qT_sb, 0, nb0)
        for st in range(ST):
            emit_v_unit(st)

        # remaining projection work rides along with qb0's attention groups:
        # (pair 0 <-> m=1 units, pair 1 <-> m=2 units)
        ride_along = {
            (0, 0): [
                lambda: emit_qk_unit(wk_sb, kT_sb, 1, 0, True),
                lambda: emit_qk_unit(wk_sb, kT_sb, 1, 2, True),
                lambda: emit_qk_unit(wq_sb, qT_sb, 1, 0, True),
                lambda: emit_qk_unit(wq_sb, qT_sb, 1, 2, True),
            ],
            (0, 1): [
                lambda: emit_qk_unit(wk_sb, kT_sb, 2, 0, True),
                lambda: emit_qk_unit(wk_sb, kT_sb, 2, 2, True),
                lambda: emit_qk_unit(wq_sb, qT_sb, 2, 0, True),
                lambda: emit_qk_unit(wq_sb, qT_sb, 2, 2, True),
            ],
        }

        for qb in range(QB):
            if qb + 1 < QB:
                mask_cache[qb + 1] = load_mask(qb + 1)
            mask_halves = mask_cache.pop(qb)
            # previous qb's output projection rides along with this qb
            proj_sts = list(range((qb - 1) * 4, qb * 4)) if qb > 0 else []

            for hp in range(3):
                av = [
                    ps_av.tile([HD + 1, QBLK], F32, tag="av", name=f"av{hp}a"),
                    ps_av.tile([HD + 1, QBLK], F32, tag="av", name=f"av{hp}b"),
                ]
                extras = ride_along.get((qb, hp), [])
                for gi, g0 in enumerate(groups):
                    emit_group(qb, hp, g0, mask_halves, av)
                    if gi % 2 == 1 and extras:
                        extras.pop(0)()
                    # qb-1's finalize chain (gpsimd bcasts + DVE muls into
                    # attn_sb) needs ~15us of slack, so projection rides with
                    # the LAST pair of this qb
                    if hp == 2 and gi % 2 == 0 and proj_sts:
                        emit_proj_unit(proj_sts.pop(0))

        while pending:
            issue_av(pending.pop(0))
        for st in range(3 * 4, 4 * 4):
            emit_proj_unit(st)

    nc.compile()
    return nc


def _prep_core_inputs(x, mask, w_qkv, w_proj, core):
    import ml_dtypes

    bf16 = ml_dtypes.bfloat16
    b, g = core // 2, core % 2
    scale = HD ** -0.5
    s0, s1 = 384 * g, 384 * (g + 1)
    def sbufize(a, t):  # [t*128, d] -> [128, t, d]
        return np.ascontiguousarray(
            a.reshape(t, 128, a.shape[-1]).transpose(1, 0, 2)
        ).astype(bf16)

    mask01T = np.array([1.0, 0.0], dtype=bf16)[mask[b].T]  # [k, q]
    # [qb, half, p, c, q] with k = half*1024 + c*128 + p
    mask_d = np.ascontiguousarray(
        mask01T.reshape(2, 8, 128, 4, 512).transpose(3, 0, 2, 1, 4)
    )
    xw = np.concatenate(
        [
            sbufize(x[b].T, 6),
            sbufize(w_qkv[C + s0 : C + s1, :].T, 6),
            sbufize((w_qkv[s0:s1, :] * scale).T, 6),
            sbufize(w_qkv[2 * C + s0 : 2 * C + s1, :].T, 6),
        ],
        axis=2,
    )
    return {
        "xw": np.ascontiguousarray(xw),
        "wproj": sbufize(w_proj[:, s0:s1].T, 3),
        "maskT": mask_d,
        "vones": np.ones((128, S // 128 * H_PER_CORE), dtype=bf16),
    }


_NC_CACHE = {}


def get_nc():
    if "nc" not in _NC_CACHE:
        _NC_CACHE["nc"] = _build_kernel()
    return _NC_CACHE["nc"]


def _build_runner(nc):
    """Reusable jitted shard_map callable over the 8 cores (mirrors
    bass2jax.run_bass_via_pjrt but cacheable across calls)."""
    import jax
    from jax.experimental.shard_map import shard_map
    from jax.sharding import Mesh, PartitionSpec

    from concourse.bass2jax import (
        _bass_exec_p,
        install_neuronx_cc_hook,
        partition_id_tensor,
    )

    install_neuronx_cc_hook()
    partition_name = nc.partition_id_tensor.name if nc.partition_id_tensor else None
    in_names, out_names, out_avals, zero_outs = [], [], [], []
    for alloc in nc.m.functions[0].allocations:
        if not isinstance(alloc, mybir.MemoryLocationSet):
            continue
        name = alloc.memorylocations[0].name
        if alloc.kind == "ExternalInput":
            if name != partition_name:
                in_names.append(name)
        elif alloc.kind == "ExternalOutput":
            out_names.append(name)
            shape = tuple(alloc.tensor_shape)
            dtype = mybir.dt.np(alloc.dtype)
            out_avals.append(jax.core.ShapedArray(shape, dtype))
            zero_outs.append(np.zeros(shape, dtype))
    n_params = len(in_names)
    all_in_names = list(in_names) + list(out_names)
    if partition_name is not None:
        all_in_names.append(partition_name)

    def _body(*args):
        operands = list(args)
        if partition_name is not None:
            operands.append(partition_id_tensor())
        outs = _bass_exec_p.bind(
            *operands,
            out_avals=tuple(out_avals),
            in_names=tuple(all_in_names),
            out_names=tuple(out_names),
            lowering_input_output_aliases=(),
            sim_require_finite=True,
            sim_require_nnan=True,
            nc=nc,
        )
        return tuple(outs)

    n_cores = nc.num_devices
    devices = jax.devices()[:n_cores]
    mesh = Mesh(np.asarray(devices), ("core",))
    in_specs = (PartitionSpec("core"),) * (n_params + len(out_names))
    out_specs = (PartitionSpec("core"),) * len(out_names)
    fn = jax.jit(
        shard_map(
            _body, mesh=mesh, in_specs=in_specs, out_specs=out_specs, check_rep=False
        ),
        keep_unused=True,
    )
    return fn, in_names, out_names, zero_outs


_RUNNER_CACHE = {}


def get_runner(nc, in_maps):
    """Return (fn, dev_args) for repeated dispatch of `nc` with `in_maps`."""
    import jax
    from jax.sharding import Mesh, NamedSharding, PartitionSpec

    key = id(nc)
    if key not in _RUNNER_CACHE:
        _RUNNER_CACHE[key] = _build_runner(nc)
    fn, in_names, out_names, zero_outs = _RUNNER_CACHE[key]
    n_cores = nc.num_devices
    mesh = Mesh(np.asarray(jax.devices()[:n_cores]), ("core",))
    shard = NamedSharding(mesh, PartitionSpec("core"))
    concat_in = [
        np.concatenate([np.asarray(in_maps[c][n]) for c in range(n_cores)], axis=0)
        for n in in_names
    ]
    dev_in = [jax.device_put(a, shard) for a in concat_in]
    zkey = ("zeros", key)
    if zkey not in _RUNNER_CACHE:
        concat_zeros = [
            np.zeros((n_cores * z.shape[0], *z.shape[1:]), z.dtype) for z in zero_outs
        ]
        _RUNNER_CACHE[zkey] = [jax.device_put(a, shard) for a in concat_zeros]
    return fn, dev_in + _RUNNER_CACHE[zkey]


def run_cached(nc, in_maps):
    """Execute via the cached runner; returns per-core result dicts."""
    fn, dev_args = get_runner(nc, in_maps)
    out_arrs = fn(*dev_args)
    _, _, out_names, zero_outs = _RUNNER_CACHE[id(nc)]
    n_cores = nc.num_devices
    fetched = [
        np.asarray(a).reshape(n_cores, *zero_outs[i].shape)
        for i, a in enumerate(out_arrs)
    ]
    return [
        {name: fetched[i][c] for i, name in enumerate(out_names)}
        for c in range(n_cores)
    ]


def make_in_maps(x, mask, w_qkv, w_proj):
    return [_prep_core_inputs(x, mask, w_qkv, w_proj, c) for c in range(N_CORES)]


def combine(results, b_proj):
    outs = []
    for b in range(B):
        outs.append(results[2 * b]["y"] + results[2 * b + 1]["y"] + b_proj[None, :])
    return np.stack(outs).astype(np.float32)


def kernel(x, mask, w_qkv, w_proj, b_proj):
    x = np.asarray(x, dtype=np.float32)
    mask = np.asarray(mask)
    w_qkv = np.asarray(w_qkv, dtype=np.float32)
    w_proj = np.asarray(w_proj, dtype=np.float32)
    b_proj = np.asarray(b_proj, dtype=np.float32)

    nc = get_nc()
    in_maps = make_in_maps(x, mask, w_qkv, w_proj)
    try:
        results = run_cached(nc, in_maps)
    except Exception:
        results = run_bass_kernel_spmd(nc, in_maps, list(range(N_CORES))).results
    return combine(results, b_proj)


# revision 29
# speedup vs baseline: 1.1628x; 1.1628x over previous
"""TRN2 Bass kernel: masked multi-head attention block (B=4, S=2048, C=768, H=12).

Sharding: 8 cores = 4 batches x 2 head-groups (6 heads each).  Each core runs a
flash-attention-style Bass/Tile kernel over its (batch, head-group) shard:

  qT/kT: [384, S] feature-major bf16 projections from xT (q pre-scaled by hd^-0.5)
  v:     [S, 6*65] bf16 natural layout, a ones column appended per head
  scoresT[k, q] = k . q (contract hd=64, head pairs row-packed on the PE array)
  pT = exp(scoresT) on the scalar engine (psum f32 -> sbuf bf16)
  pT *= mask01T on DVE (bf16 2x mode); mask01 = 1 - mask, so masked keys get
  exactly 0 (reference adds -1e5 pre-softmax: exp(s-1e5) ~ 0)
  avT[65, 512] accumulated over key chunks; row 64 = softmax denominator
  attn_outT = avT[0:64] * recip(denominator)  (partition-broadcast on gpsimd)
  y = attn_outT.T @ w_projT slice  (row-parallel output projection)

Pipelining: one PSUM ring ([128,2,512] slots) is shared by the qkv-projection
units, the score groups, and the output-projection units, so all three phases
interleave in the PE stream: attention on (qb0, pair0) starts as soon as the
m=0 q/k slices and v are projected, the remaining q/k slices ride along with
qb0's score groups, and each query block's output projection + DMA rides along
with the next query block.  AV matmuls lag the score matmuls by 2 groups so
the PE never waits on the exp/mask chain (keeps the HAM p-state warm).
Host-side: transposes/slices the weights per core, builds the bf16 {0,1} mask,
sums the two per-batch partials, and adds b_proj.
"""

from contextlib import ExitStack

import numpy as np

import concourse.tile as tile
from concourse import bacc, mybir
from concourse.bass_utils import run_bass_kernel_spmd

F32 = mybir.dt.float32
BF16 = mybir.dt.bfloat16

B, S, C, H = 4, 2048, 768, 12
HD = 64
H_PER_CORE = 6
D_CORE = H_PER_CORE * HD  # 384
QBLK = 512
GRP = 2
N_CORES = 8
AV_LAG = 2


def _build_kernel():
    nc = bacc.Bacc(
        trn_type="TRN2", target_bir_lowering=False, debug=False, num_devices=N_CORES
    )
    KC = S // 128  # 16 key chunks
    QB = S // QBLK  # 4 query blocks
    ST = S // 128
    groups = []
    for half in range(KC // 8):
        base = half * 8
        for g0 in (0, 2, 4, 6):
            groups.append(base + g0)

    # all inputs pre-arranged host-side into the exact SBUF layout so every
    # DMA is 128 big contiguous per-partition descriptors
    xw = nc.dram_tensor(
        "xw", [128, 6, S + 3 * D_CORE], BF16, kind="ExternalInput"
    ).ap()
    wproj = nc.dram_tensor("wproj", [128, 3, C], BF16, kind="ExternalInput").ap()
    vones = nc.dram_tensor("vones", [128, S // 128 * H_PER_CORE], BF16, kind="ExternalInput").ap()
    maskT = nc.dram_tensor(
        "maskT", [S // QBLK, KC // 8, 128, 8, QBLK], BF16, kind="ExternalInput"
    ).ap()
    y = nc.dram_tensor("y", [S, C], F32, kind="ExternalOutput").ap()

    with tile.TileContext(nc) as tc, ExitStack() as ctx:
        consts = ctx.enter_context(tc.tile_pool(name="consts", bufs=1))
        qkv_pool = ctx.enter_context(tc.tile_pool(name="qkv", bufs=1))
        mpool = ctx.enter_context(tc.tile_pool(name="mask", bufs=4))
        xpool = ctx.enter_context(tc.tile_pool(name="x1", bufs=1))
        ppool = ctx.enter_context(tc.tile_pool(name="pT", bufs=8))
        dpool = ctx.enter_context(tc.tile_pool(name="div", bufs=2))
        bpool = ctx.enter_context(tc.tile_pool(name="bcast", bufs=2))
        ypool = ctx.enter_context(tc.tile_pool(name="y", bufs=3))
        ps = ctx.enter_context(tc.tile_pool(name="ps", bufs=3, space="PSUM"))
        ps_av = ctx.enter_context(tc.tile_pool(name="ps_av", bufs=2, space="PSUM"))

        # ---- input DMAs: x+weights fused into one tile, 3 big transfers
        xw_sb = xpool.tile([128, 6, S + 3 * D_CORE], BF16)
        xT_sb = xw_sb[:, :, :S]
        wk_sb = xw_sb[:, :, S : S + D_CORE]
        wq_sb = xw_sb[:, :, S + D_CORE : S + 2 * D_CORE]
        wv_sb = xw_sb[:, :, S + 2 * D_CORE :]
        nc.sync.dma_start(
            xw_sb[:, :, S : S + D_CORE], xw[:, :, S : S + D_CORE]
        )
        nc.sync.dma_start(xw_sb[:, :, : 2 * QBLK], xw[:, :, : 2 * QBLK])
        nc.sync.dma_start(xw_sb[:, :, S + D_CORE :], xw[:, :, S + D_CORE :])
        nc.sync.dma_start(xw_sb[:, :, 2 * QBLK : S], xw[:, :, 2 * QBLK : S])
        wproj_sb = consts.tile([128, 3, C], BF16)
        nc.sync.dma_start(wproj_sb[:], wproj[:])

        qT_sb = qkv_pool.tile([128, 3, S], BF16)
        kT_sb = qkv_pool.tile([128, 3, S], BF16)
        vaug_sb = qkv_pool.tile([128, ST, H_PER_CORE * (HD + 1)], BF16)
        attn_sb = qkv_pool.tile([128, 3, S], BF16)

        vaug_ones = vaug_sb.rearrange("p st (h u) -> p st h u", u=HD + 1)[:, :, :, HD]
        nc.sync.dma_start(
            vaug_ones, vones.rearrange("p (st h) -> p st h", h=H_PER_CORE)
        )

        mask_cache = {}

        def load_mask(qb_i):
            halves = []
            for half in range(KC // 8):
                mh = mpool.tile([128, 8, QBLK], BF16, tag="mask", name="mask_h")
                nc.sync.dma_start(mh[:], maskT[qb_i, half])
                halves.append(mh)
            return halves

        mask_cache[0] = load_mask(0)

        # ---- unit emitters, all drawing PSUM from the shared ring ----
        qk_cp = [0]

        def emit_qk_unit(w_sb, dst, m, nb0, dve_copy=False):
            """Project two q-blocks of q or k for weight column block m."""
            u = ps.tile([128, GRP, QBLK], F32, tag="sc", name="psqk")
            for j in range(2):
                for k in range(6):
                    nc.tensor.matmul(
                        u[:, j, :],
                        w_sb[:, k, m * 128 : (m + 1) * 128],
                        xT_sb[:, k, (nb0 + j) * QBLK : (nb0 + j + 1) * QBLK],
                        start=(k == 0),
                        stop=(k == 5),
                    )
            dst_ap = dst[:, m, nb0 * QBLK : (nb0 + 2) * QBLK].rearrange(
                "p (j q) -> p j q", j=2
            )
            if dve_copy or qk_cp[0] % 2 == 0:
                nc.vector.tensor_copy(dst_ap, u[:])
            else:
                nc.scalar.copy(dst_ap, u[:])
            qk_cp[0] += 1

        def emit_v_unit(st):
            u = ps.tile([128, GRP, QBLK], F32, tag="sc", name="psv")
            pv = u[:, 0, :D_CORE]
            for k in range(6):
                nc.tensor.matmul(
                    pv,
                    xT_sb[:, k, st * 128 : (st + 1) * 128],
                    wv_sb[:, k, :],
                    start=(k == 0),
                    stop=(k == 5),
                )
            for h in range(H_PER_CORE):
                dst = vaug_sb[:, st, h * (HD + 1) : h * (HD + 1) + HD]
                src = u[:, 0, h * HD : (h + 1) * HD]
                if h % 2 == 0:
                    nc.vector.tensor_copy(dst, src)
                else:
                    nc.scalar.copy(dst, src)

        y_r = y.rearrange("(st p) o -> st p o", p=128)

        def emit_proj_unit(st):
            u = ps.tile([128, GRP, QBLK], F32, tag="sc", name="psy")
            y_sb = ypool.tile([128, C], F32, tag="ysb", name="y_sb")
            for nb2 in range(2):
                for k3 in range(3):
                    nc.tensor.matmul(
                        u[:, nb2, :384],
                        attn_sb[:, k3, st * 128 : (st + 1) * 128],
                        wproj_sb[:, k3, nb2 * 384 : (nb2 + 1) * 384],
                        start=(k3 == 0),
                        stop=(k3 == 2),
                    )
            nc.vector.tensor_copy(
                y_sb.rearrange("p (j o) -> p j o", j=2), u[:, :, :384]
            )
            nc.sync.dma_start(y_r[st], y_sb[:])

        # ---- attention machinery ----
        pending = []

        def issue_av(ent):
            qb, hp, g0, av, pT_pair = ent
            for i, h in ((0, 2 * hp), (1, 2 * hp + 1)):
                for c in range(GRP):
                    kc = g0 + c
                    nc.tensor.matmul(
                        av[i][:],
                        vaug_sb[:, kc, h * (HD + 1) : (h + 1) * (HD + 1)],
                        pT_pair[i][:, c, :],
                        start=(kc == 0),
                        stop=(kc == KC - 1),
                    )
            if g0 + GRP == KC:
                finalize_pair(qb, hp, av)

        def finalize_pair(qb, hp, av):
            av_all = dpool.tile(
                [HD + 1, 2, QBLK], F32, tag="av_all", name="av_all", bufs=2
            )
            for i in (0, 1):
                nc.vector.tensor_copy(av_all[:, i, :], av[i][:])
            dstack = dpool.tile([2, QBLK], F32, tag="dstack", name="dstack", bufs=2)
            recip = dpool.tile([2, QBLK], F32, tag="recip", name="recip", bufs=2)
            nc.gpsimd.dma_start(dstack[:], av_all[HD : HD + 1, :, :])
            nc.vector.reciprocal_approx_fast(recip[:], dstack[:])
            r2 = bpool.tile([1, 2, QBLK], F32, tag="r2", name="r2", bufs=2)
            nc.gpsimd.dma_start(r2[:], recip[:])
            tmp = bpool.tile([HD, QBLK], BF16, tag="tmp", name="tmp", bufs=2)
            bcs = []
            for i in (0, 1):
                bc = bpool.tile([HD, QBLK], F32, tag="bc", name="bc", bufs=6)
                nc.gpsimd.partition_broadcast(bc[:], r2[:, i, :])
                bcs.append(bc)
            qsl = slice(qb * QBLK, (qb + 1) * QBLK)
            nc.vector.tensor_mul(
                attn_sb[:HD, hp, qsl], av_all[:HD, 0, :], bcs[0][:]
            )
            nc.vector.tensor_mul(tmp[:], av_all[:HD, 1, :], bcs[1][:])
            nc.sync.dma_start(attn_sb[HD:128, hp, qsl], tmp[:])

        def emit_group(qb, hp, g0, mask_halves, av):
            hA, hB = 2 * hp, 2 * hp + 1
            mh = mask_halves[g0 // 8]
            moff = g0 % 8
            sc = [
                ps.tile([128, GRP, QBLK], F32, tag="sc", name="scA")[:],
                ps.tile([128, GRP, QBLK], F32, tag="sc", name="scB")[:],
            ]
            for i, h in ((0, hA), (1, hB)):
                row0 = (h % 2) * HD
                for c in range(GRP):
                    kc = g0 + c
                    nc.tensor.matmul(
                        sc[i][:, c, :],
                        kT_sb[row0 : row0 + HD, h // 2, kc * 128 : (kc + 1) * 128],
                        qT_sb[row0 : row0 + HD, h // 2, qb * QBLK : (qb + 1) * QBLK],
                        start=True,
                        stop=True,
                        tile_position=(row0, 0),
                    )
            pT_views = []
            for i in (0, 1):
                pT = ppool.tile(
                    [128, GRP, QBLK], BF16, tag="pT2", name="pT2", bufs=4
                )
                nc.scalar.activation(
                    pT[:], sc[i], mybir.ActivationFunctionType.Exp
                )
                pT_views.append(pT[:])
            pT_pair = []
            for i in (0, 1):
                pTm = ppool.tile([128, GRP, QBLK], BF16, tag="pTm", name="pTm", bufs=10)
                nc.vector.tensor_mul(pTm[:], pT_views[i], mh[:, moff : moff + GRP, :])
                pT_pair.append(pTm)
            pending.append((qb, hp, g0, av, pT_pair))
            if len(pending) > AV_LAG:
                issue_av(pending.pop(0))

        # ---- schedule ----
        # startup: project m=0 slices of k and q, then all of v
        for nb0 in (0, 2):
            emit_qk_unit(wk_sb, kT_sb, 0, nb0)
        emit_qk_unit(wq_sb, qT_sb, 0, 0)
        for st in range(ST):
            emit_v_unit(st)

        # remaining projection work rides along with qb0's attention groups:
        # (pair 0 <-> m=1 units, pair 1 <-> m=2 units)
        ride_along = {
            (0, 0): [
                lambda: emit_qk_unit(wk_sb, kT_sb, 1, 0, True),
                lambda: emit_qk_unit(wk_sb, kT_sb, 1, 2, True),
                lambda: emit_qk_unit(wq_sb, qT_sb, 1, 0, True),
            ],
            (0, 1): [
                lambda: emit_qk_unit(wk_sb, kT_sb, 2, 0, True),
                lambda: emit_qk_unit(wk_sb, kT_sb, 2, 2, True),
                lambda: emit_qk_unit(wq_sb, qT_sb, 2, 0, True),
            ],
            (1, 0): [
                lambda: emit_qk_unit(wq_sb, qT_sb, 0, 2, True),
                lambda: emit_qk_unit(wq_sb, qT_sb, 1, 2, True),
            ],
            (1, 1): [
                lambda: emit_qk_unit(wq_sb, qT_sb, 2, 2, True),
            ],
        }

        for qb in range(QB):
            if qb + 1 < QB:
                mask_cache[qb + 1] = load_mask(qb + 1)
            mask_halves = mask_cache.pop(qb)
            # previous qb's output projection rides along with this qb
            proj_sts = list(range((qb - 1) * 4, qb * 4)) if qb > 0 else []

            for hp in range(3):
                av = [
                    ps_av.tile([HD + 1, QBLK], F32, tag="av", name=f"av{hp}a"),
                    ps_av.tile([HD + 1, QBLK], F32, tag="av", name=f"av{hp}b"),
                ]
                extras = ride_along.get((qb, hp), [])
                for gi, g0 in enumerate(groups):
                    emit_group(qb, hp, g0, mask_halves, av)
                    if gi % 2 == 1 and extras:
                        extras.pop(0)()
                    # qb-1's finalize chain (gpsimd bcasts + DVE muls into
                    # attn_sb) needs ~15us of slack, so projection rides with
                    # the LAST pair of this qb
                    if hp == 2 and gi % 2 == 0 and proj_sts:
                        emit_proj_unit(proj_sts.pop(0))

        while pending:
            issue_av(pending.pop(0))
        for st in range(3 * 4, 4 * 4):
            emit_proj_unit(st)

    nc.compile()
    return nc


def _prep_core_inputs(x, mask, w_qkv, w_proj, core):
    import ml_dtypes

    bf16 = ml_dtypes.bfloat16
    b, g = core // 2, core % 2
    scale = HD ** -0.5
    s0, s1 = 384 * g, 384 * (g + 1)
    def sbufize(a, t):  # [t*128, d] -> [128, t, d]
        return np.ascontiguousarray(
            a.reshape(t, 128, a.shape[-1]).transpose(1, 0, 2)
        ).astype(bf16)

    mask01T = np.array([1.0, 0.0], dtype=bf16)[mask[b].T]  # [k, q]
    # [qb, half, p, c, q] with k = half*1024 + c*128 + p
    mask_d = np.ascontiguousarray(
        mask01T.reshape(2, 8, 128, 4, 512).transpose(3, 0, 2, 1, 4)
    )
    xw = np.concatenate(
        [
            sbufize(x[b].T, 6),
            sbufize(w_qkv[C + s0 : C + s1, :].T, 6),
            sbufize((w_qkv[s0:s1, :] * scale).T, 6),
            sbufize(w_qkv[2 * C + s0 : 2 * C + s1, :].T, 6),
        ],
        axis=2,
    )
    return {
        "xw": np.ascontiguousarray(xw),
        "wproj": sbufize(w_proj[:, s0:s1].T, 3),
        "maskT": mask_d,
        "vones": np.ones((128, S // 128 * H_PER_CORE), dtype=bf16),
    }


_NC_CACHE = {}


def get_nc():
    if "nc" not in _NC_CACHE:
        _NC_CACHE["nc"] = _build_kernel()
    return _NC_CACHE["nc"]


def _build_runner(nc):
    """Reusable jitted shard_map callable over the 8 cores (mirrors
    bass2jax.run_bass_via_pjrt but cacheable across calls)."""
    import jax
    from jax.experimental.shard_map import shard_map
    from jax.sharding import Mesh, PartitionSpec

    from concourse.bass2jax import (
        _bass_exec_p,
        install_neuronx_cc_hook,
        partition_id_tensor,
    )

    install_neuronx_cc_hook()
    partition_name = nc.partition_id_tensor.name if nc.partition_id_tensor else None
    in_names, out_names, out_avals, zero_outs = [], [], [], []
    for alloc in nc.m.functions[0].allocations:
        if not isinstance(alloc, mybir.MemoryLocationSet):
            continue
        name = alloc.memorylocations[0].name
        if alloc.kind == "ExternalInput":
            if name != partition_name:
                in_names.append(name)
        elif alloc.kind == "ExternalOutput":
            out_names.append(name)
            shape = tuple(alloc.tensor_shape)
            dtype = mybir.dt.np(alloc.dtype)
            out_avals.append(jax.core.ShapedArray(shape, dtype))
            zero_outs.append(np.zeros(shape, dtype))
    n_params = len(in_names)
    all_in_names = list(in_names) + list(out_names)
    if partition_name is not None:
        all_in_names.append(partition_name)

    def _body(*args):
        operands = list(args)
        if partition_name is not None:
            operands.append(partition_id_tensor())
        outs = _bass_exec_p.bind(
            *operands,
            out_avals=tuple(out_avals),
            in_names=tuple(all_in_names),
            out_names=tuple(out_names),
            lowering_input_output_aliases=(),
            sim_require_finite=True,
            sim_require_nnan=True,
            nc=nc,
        )
        return tuple(outs)

    n_cores = nc.num_devices
    devices = jax.devices()[:n_cores]
    mesh = Mesh(np.asarray(devices), ("core",))
    in_specs = (PartitionSpec("core"),) * (n_params + len(out_names))
    out_specs = (PartitionSpec("core"),) * len(out_names)
    fn = jax.jit(
        shard_map(
            _body, mesh=mesh, in_specs=in_specs, out_specs=out_specs, check_rep=False
        ),
        keep_unused=True,
    )
    return fn, in_names, out_names, zero_outs


_RUNNER_CACHE = {}


def get_runner(nc, in_maps):
    """Return (fn, dev_args) for repeated dispatch of `nc` with `in_maps`."""
    import jax
    from jax.sharding import Mesh, NamedSharding, PartitionSpec

    key = id(nc)
    if key not in _RUNNER_CACHE:
        _RUNNER_CACHE[key] = _build_runner(nc)
    fn, in_names, out_names, zero_outs = _RUNNER_CACHE[key]
    n_cores = nc.num_devices
    mesh = Mesh(np.asarray(jax.devices()[:n_cores]), ("core",))
    shard = NamedSharding(mesh, PartitionSpec("core"))
    concat_in = [
        np.concatenate([np.asarray(in_maps[c][n]) for c in range(n_cores)], axis=0)
        for n in in_names
    ]
    dev_in = [jax.device_put(a, shard) for a in concat_in]
    zkey = ("zeros", key)
    if zkey not in _RUNNER_CACHE:
        concat_zeros = [
            np.zeros((n_cores * z.shape[0], *z.shape[1:]), z.dtype) for z in zero_outs
        ]
        _RUNNER_CACHE[zkey] = [jax.device_put(a, shard) for a in concat_zeros]
    return fn, dev_in + _RUNNER_CACHE[zkey]


def run_cached(nc, in_maps):
    """Execute via the cached runner; returns per-core result dicts."""
    fn, dev_args = get_runner(nc, in_maps)
    out_arrs = fn(*dev_args)
    _, _, out_names, zero_outs = _RUNNER_CACHE[id(nc)]
    n_cores = nc.num_devices
    fetched = [
        np.asarray(a).reshape(n_cores, *zero_outs[i].shape)
        for i, a in enumerate(out_arrs)
    ]
    return [
        {name: fetched[i][c] for i, name in enumerate(out_names)}
        for c in range(n_cores)
    ]


def make_in_maps(x, mask, w_qkv, w_proj):
    return [_prep_core_inputs(x, mask, w_qkv, w_proj, c) for c in range(N_CORES)]


def combine(results, b_proj):
    outs = []
    for b in range(B):
        outs.append(results[2 * b]["y"] + results[2 * b + 1]["y"] + b_proj[None, :])
    return np.stack(outs).astype(np.float32)


def kernel(x, mask, w_qkv, w_proj, b_proj):
    x = np.asarray(x, dtype=np.float32)
    mask = np.asarray(mask)
    w_qkv = np.asarray(w_qkv, dtype=np.float32)
    w_proj = np.asarray(w_proj, dtype=np.float32)
    b_proj = np.asarray(b_proj, dtype=np.float32)

    nc = get_nc()
    in_maps = make_in_maps(x, mask, w_qkv, w_proj)
    try:
        results = run_cached(nc, in_maps)
    except Exception:
        results = run_bass_kernel_spmd(nc, in_maps, list(range(N_CORES))).results
    return combine(results, b_proj)


# revision 30
# speedup vs baseline: 1.1807x; 1.0154x over previous
"""TRN2 Bass kernel: masked multi-head attention block (B=4, S=2048, C=768, H=12).

Sharding: 8 cores = 4 batches x 2 head-groups (6 heads each).  Each core runs a
flash-attention-style Bass/Tile kernel over its (batch, head-group) shard:

  qT/kT: [384, S] feature-major bf16 projections from xT (q pre-scaled by hd^-0.5)
  v:     [S, 6*65] bf16 natural layout, a ones column appended per head
  scoresT[k, q] = k . q (contract hd=64, head pairs row-packed on the PE array)
  pT = exp(scoresT) on the scalar engine (psum f32 -> sbuf bf16)
  pT *= mask01T on DVE (bf16 2x mode); mask01 = 1 - mask, so masked keys get
  exactly 0 (reference adds -1e5 pre-softmax: exp(s-1e5) ~ 0)
  avT[65, 512] accumulated over key chunks; row 64 = softmax denominator
  attn_outT = avT[0:64] * recip(denominator)  (partition-broadcast on gpsimd)
  y = attn_outT.T @ w_projT slice  (row-parallel output projection)

Pipelining: one PSUM ring ([128,2,512] slots) is shared by the qkv-projection
units, the score groups, and the output-projection units, so all three phases
interleave in the PE stream: attention on (qb0, pair0) starts as soon as the
m=0 q/k slices and v are projected, the remaining q/k slices ride along with
qb0's score groups, and each query block's output projection + DMA rides along
with the next query block.  AV matmuls lag the score matmuls by 2 groups so
the PE never waits on the exp/mask chain (keeps the HAM p-state warm).
Host-side: transposes/slices the weights per core, builds the bf16 {0,1} mask,
sums the two per-batch partials, and adds b_proj.
"""

from contextlib import ExitStack

import numpy as np

import concourse.tile as tile
from concourse import bacc, mybir
from concourse.bass_utils import run_bass_kernel_spmd

F32 = mybir.dt.float32
BF16 = mybir.dt.bfloat16

B, S, C, H = 4, 2048, 768, 12
HD = 64
H_PER_CORE = 6
D_CORE = H_PER_CORE * HD  # 384
QBLK = 512
GRP = 2
N_CORES = 8
AV_LAG = 2


def _build_kernel():
    nc = bacc.Bacc(
        trn_type="TRN2", target_bir_lowering=False, debug=False, num_devices=N_CORES
    )
    KC = S // 128  # 16 key chunks
    QB = S // QBLK  # 4 query blocks
    ST = S // 128
    groups = []
    for half in range(KC // 8):
        base = half * 8
        for g0 in (0, 2, 4, 6):
            groups.append(base + g0)

    # all inputs pre-arranged host-side into the exact SBUF layout so every
    # DMA is 128 big contiguous per-partition descriptors
    xw = nc.dram_tensor(
        "xw", [128, 6, S + 3 * D_CORE], BF16, kind="ExternalInput"
    ).ap()
    wproj = nc.dram_tensor("wproj", [128, 3, C], BF16, kind="ExternalInput").ap()
    vones = nc.dram_tensor("vones", [128, S // 128 * H_PER_CORE], BF16, kind="ExternalInput").ap()
    maskT = nc.dram_tensor(
        "maskT", [S // QBLK, KC // 8, 128, 8, QBLK], BF16, kind="ExternalInput"
    ).ap()
    y = nc.dram_tensor("y", [S, C], F32, kind="ExternalOutput").ap()

    with tile.TileContext(nc) as tc, ExitStack() as ctx:
        consts = ctx.enter_context(tc.tile_pool(name="consts", bufs=1))
        qkv_pool = ctx.enter_context(tc.tile_pool(name="qkv", bufs=1))
        mpool = ctx.enter_context(tc.tile_pool(name="mask", bufs=4))
        xpool = ctx.enter_context(tc.tile_pool(name="x1", bufs=1))
        ppool = ctx.enter_context(tc.tile_pool(name="pT", bufs=8))
        dpool = ctx.enter_context(tc.tile_pool(name="div", bufs=2))
        bpool = ctx.enter_context(tc.tile_pool(name="bcast", bufs=2))
        ypool = ctx.enter_context(tc.tile_pool(name="y", bufs=3))
        ps = ctx.enter_context(tc.tile_pool(name="ps", bufs=3, space="PSUM"))
        ps_av = ctx.enter_context(tc.tile_pool(name="ps_av", bufs=2, space="PSUM"))

        # ---- input DMAs: x+weights fused into one tile, 3 big transfers
        xw_sb = xpool.tile([128, 6, S + 3 * D_CORE], BF16)
        xT_sb = xw_sb[:, :, :S]
        wk_sb = xw_sb[:, :, S : S + D_CORE]
        wq_sb = xw_sb[:, :, S + D_CORE : S + 2 * D_CORE]
        wv_sb = xw_sb[:, :, S + 2 * D_CORE :]
        nc.sync.dma_start(
            xw_sb[:, :, S : S + D_CORE], xw[:, :, S : S + D_CORE]
        )
        nc.sync.dma_start(xw_sb[:, :, : 2 * QBLK], xw[:, :, : 2 * QBLK])
        nc.sync.dma_start(xw_sb[:, :, S + D_CORE :], xw[:, :, S + D_CORE :])
        nc.sync.dma_start(xw_sb[:, :, 2 * QBLK : S], xw[:, :, 2 * QBLK : S])
        wproj_sb = consts.tile([128, 3, C], BF16)
        nc.sync.dma_start(wproj_sb[:], wproj[:])

        qT_sb = qkv_pool.tile([128, 3, S], BF16)
        kT_sb = qkv_pool.tile([128, 3, S], BF16)
        vaug_sb = qkv_pool.tile([128, ST, H_PER_CORE * (HD + 1)], BF16)
        attn_sb = qkv_pool.tile([128, 3, S], BF16)

        vaug_ones = vaug_sb.rearrange("p st (h u) -> p st h u", u=HD + 1)[:, :, :, HD]
        nc.sync.dma_start(
            vaug_ones, vones.rearrange("p (st h) -> p st h", h=H_PER_CORE)
        )

        mask_cache = {}

        def load_mask(qb_i):
            halves = []
            for half in range(KC // 8):
                mh = mpool.tile([128, 8, QBLK], BF16, tag="mask", name="mask_h")
                nc.sync.dma_start(mh[:], maskT[qb_i, half])
                halves.append(mh)
            return halves

        mask_cache[0] = load_mask(0)

        # ---- unit emitters, all drawing PSUM from the shared ring ----
        qk_cp = [0]

        def emit_qk_unit(w_sb, dst, m, nb0, dve_copy=False):
            """Project two q-blocks of q or k for weight column block m."""
            u = ps.tile([128, GRP, QBLK], F32, tag="sc", name="psqk")
            for j in range(2):
                for k in range(6):
                    nc.tensor.matmul(
                        u[:, j, :],
                        w_sb[:, k, m * 128 : (m + 1) * 128],
                        xT_sb[:, k, (nb0 + j) * QBLK : (nb0 + j + 1) * QBLK],
                        start=(k == 0),
                        stop=(k == 5),
                    )
            dst_ap = dst[:, m, nb0 * QBLK : (nb0 + 2) * QBLK].rearrange(
                "p (j q) -> p j q", j=2
            )
            if dve_copy or qk_cp[0] % 2 == 0:
                nc.vector.tensor_copy(dst_ap, u[:])
            else:
                nc.scalar.copy(dst_ap, u[:])
            qk_cp[0] += 1

        def emit_v_unit(st):
            u = ps.tile([128, GRP, QBLK], F32, tag="sc", name="psv")
            pv = u[:, 0, :D_CORE]
            for k in range(6):
                nc.tensor.matmul(
                    pv,
                    xT_sb[:, k, st * 128 : (st + 1) * 128],
                    wv_sb[:, k, :],
                    start=(k == 0),
                    stop=(k == 5),
                )
            for h in range(H_PER_CORE):
                dst = vaug_sb[:, st, h * (HD + 1) : h * (HD + 1) + HD]
                src = u[:, 0, h * HD : (h + 1) * HD]
                if h % 2 == 0:
                    nc.vector.tensor_copy(dst, src)
                else:
                    nc.scalar.copy(dst, src)

        y_r = y.rearrange("(st p) o -> st p o", p=128)

        def emit_proj_unit(st):
            u = ps.tile([128, GRP, QBLK], F32, tag="sc", name="psy")
            y_sb = ypool.tile([128, C], F32, tag="ysb", name="y_sb")
            for nb2 in range(2):
                for k3 in range(3):
                    nc.tensor.matmul(
                        u[:, nb2, :384],
                        attn_sb[:, k3, st * 128 : (st + 1) * 128],
                        wproj_sb[:, k3, nb2 * 384 : (nb2 + 1) * 384],
                        start=(k3 == 0),
                        stop=(k3 == 2),
                    )
            nc.vector.tensor_copy(
                y_sb.rearrange("p (j o) -> p j o", j=2), u[:, :, :384]
            )
            nc.sync.dma_start(y_r[st], y_sb[:])

        # ---- attention machinery ----
        pending = []

        def issue_av(ent):
            qb, hp, g0, av, pT_pair = ent
            for i, h in ((0, 2 * hp), (1, 2 * hp + 1)):
                for c in range(GRP):
                    kc = g0 + c
                    nc.tensor.matmul(
                        av[i][:],
                        vaug_sb[:, kc, h * (HD + 1) : (h + 1) * (HD + 1)],
                        pT_pair[i][:, c, :],
                        start=(kc == 0),
                        stop=(kc == KC - 1),
                    )
            if g0 + GRP == KC:
                finalize_pair(qb, hp, av)

        def finalize_pair(qb, hp, av):
            av_all = dpool.tile(
                [HD + 1, 2, QBLK], F32, tag="av_all", name="av_all", bufs=2
            )
            for i in (0, 1):
                nc.vector.tensor_copy(av_all[:, i, :], av[i][:])
            dstack = dpool.tile([2, QBLK], F32, tag="dstack", name="dstack", bufs=2)
            recip = dpool.tile([2, QBLK], F32, tag="recip", name="recip", bufs=2)
            nc.sync.dma_start(dstack[:], av_all[HD : HD + 1, :, :])
            nc.vector.reciprocal_approx_fast(recip[:], dstack[:])
            r2 = bpool.tile([1, 2, QBLK], F32, tag="r2", name="r2", bufs=2)
            nc.sync.dma_start(r2[:], recip[:])
            tmp = bpool.tile([HD, QBLK], BF16, tag="tmp", name="tmp", bufs=2)
            bcs = []
            for i in (0, 1):
                bc = bpool.tile([HD, QBLK], F32, tag="bc", name="bc", bufs=6)
                nc.gpsimd.partition_broadcast(bc[:], r2[:, i, :])
                bcs.append(bc)
            qsl = slice(qb * QBLK, (qb + 1) * QBLK)
            nc.vector.tensor_mul(
                attn_sb[:HD, hp, qsl], av_all[:HD, 0, :], bcs[0][:]
            )
            nc.vector.tensor_mul(tmp[:], av_all[:HD, 1, :], bcs[1][:])
            nc.sync.dma_start(attn_sb[HD:128, hp, qsl], tmp[:])

        def emit_group(qb, hp, g0, mask_halves, av):
            hA, hB = 2 * hp, 2 * hp + 1
            mh = mask_halves[g0 // 8]
            moff = g0 % 8
            sc = [
                ps.tile([128, GRP, QBLK], F32, tag="sc", name="scA")[:],
                ps.tile([128, GRP, QBLK], F32, tag="sc", name="scB")[:],
            ]
            for i, h in ((0, hA), (1, hB)):
                row0 = (h % 2) * HD
                for c in range(GRP):
                    kc = g0 + c
                    nc.tensor.matmul(
                        sc[i][:, c, :],
                        kT_sb[row0 : row0 + HD, h // 2, kc * 128 : (kc + 1) * 128],
                        qT_sb[row0 : row0 + HD, h // 2, qb * QBLK : (qb + 1) * QBLK],
                        start=True,
                        stop=True,
                        tile_position=(row0, 0),
                    )
            pT_views = []
            for i in (0, 1):
                pT = ppool.tile(
                    [128, GRP, QBLK], BF16, tag="pT2", name="pT2", bufs=4
                )
                nc.scalar.activation(
                    pT[:], sc[i], mybir.ActivationFunctionType.Exp
                )
                pT_views.append(pT[:])
            pT_pair = []
            for i in (0, 1):
                pTm = ppool.tile([128, GRP, QBLK], BF16, tag="pTm", name="pTm", bufs=10)
                nc.vector.tensor_mul(pTm[:], pT_views[i], mh[:, moff : moff + GRP, :])
                pT_pair.append(pTm)
            pending.append((qb, hp, g0, av, pT_pair))
            if len(pending) > AV_LAG:
                issue_av(pending.pop(0))

        # ---- schedule ----
        # startup: project m=0 slices of k and q, then all of v
        for nb0 in (0, 2):
            emit_qk_unit(wk_sb, kT_sb, 0, nb0)
        emit_qk_unit(wq_sb, qT_sb, 0, 0)
        for st in range(ST):
            emit_v_unit(st)

        # remaining projection work rides along with qb0's attention groups:
        # (pair 0 <-> m=1 units, pair 1 <-> m=2 units)
        ride_along = {
            (0, 0): [
                lambda: emit_qk_unit(wk_sb, kT_sb, 1, 0, True),
                lambda: emit_qk_unit(wk_sb, kT_sb, 1, 2, True),
                lambda: emit_qk_unit(wq_sb, qT_sb, 1, 0, True),
            ],
            (0, 1): [
                lambda: emit_qk_unit(wk_sb, kT_sb, 2, 0, True),
                lambda: emit_qk_unit(wk_sb, kT_sb, 2, 2, True),
                lambda: emit_qk_unit(wq_sb, qT_sb, 2, 0, True),
            ],
            (1, 0): [
                lambda: emit_qk_unit(wq_sb, qT_sb, 0, 2, True),
                lambda: emit_qk_unit(wq_sb, qT_sb, 1, 2, True),
            ],
            (1, 1): [
                lambda: emit_qk_unit(wq_sb, qT_sb, 2, 2, True),
            ],
        }

        for qb in range(QB):
            if qb + 1 < QB:
                mask_cache[qb + 1] = load_mask(qb + 1)
            mask_halves = mask_cache.pop(qb)
            # previous qb's output projection rides along with this qb
            proj_sts = list(range((qb - 1) * 4, qb * 4)) if qb > 0 else []

            for hp in range(3):
                av = [
                    ps_av.tile([HD + 1, QBLK], F32, tag="av", name=f"av{hp}a"),
                    ps_av.tile([HD + 1, QBLK], F32, tag="av", name=f"av{hp}b"),
                ]
                extras = ride_along.get((qb, hp), [])
                for gi, g0 in enumerate(groups):
                    emit_group(qb, hp, g0, mask_halves, av)
                    if gi % 2 == 1 and extras:
                        extras.pop(0)()
                    # qb-1's finalize chain (gpsimd bcasts + DVE muls into
                    # attn_sb) needs ~15us of slack, so projection rides with
                    # the LAST pair of this qb
                    if hp == 2 and gi % 2 == 0 and proj_sts:
                        emit_proj_unit(proj_sts.pop(0))

        while pending:
            issue_av(pending.pop(0))
        for st in range(3 * 4, 4 * 4):
            emit_proj_unit(st)

    nc.compile()
    return nc


def _prep_core_inputs(x, mask, w_qkv, w_proj, core):
    import ml_dtypes

    bf16 = ml_dtypes.bfloat16
    b, g = core // 2, core % 2
    scale = HD ** -0.5
    s0, s1 = 384 * g, 384 * (g + 1)
    def sbufize(a, t):  # [t*128, d] -> [128, t, d]
        return np.ascontiguousarray(
            a.reshape(t, 128, a.shape[-1]).transpose(1, 0, 2)
        ).astype(bf16)

    mask01T = np.array([1.0, 0.0], dtype=bf16)[mask[b].T]  # [k, q]
    # [qb, half, p, c, q] with k = half*1024 + c*128 + p
    mask_d = np.ascontiguousarray(
        mask01T.reshape(2, 8, 128, 4, 512).transpose(3, 0, 2, 1, 4)
    )
    xw = np.concatenate(
        [
            sbufize(x[b].T, 6),
            sbufize(w_qkv[C + s0 : C + s1, :].T, 6),
            sbufize((w_qkv[s0:s1, :] * scale).T, 6),
            sbufize(w_qkv[2 * C + s0 : 2 * C + s1, :].T, 6),
        ],
        axis=2,
    )
    return {
        "xw": np.ascontiguousarray(xw),
        "wproj": sbufize(w_proj[:, s0:s1].T, 3),
        "maskT": mask_d,
        "vones": np.ones((128, S // 128 * H_PER_CORE), dtype=bf16),
    }


_NC_CACHE = {}


def get_nc():
    if "nc" not in _NC_CACHE:
        _NC_CACHE["nc"] = _build_kernel()
    return _NC_CACHE["nc"]


def _build_runner(nc):
    """Reusable jitted shard_map callable over the 8 cores (mirrors
    bass2jax.run_bass_via_pjrt but cacheable across calls)."""
    import jax
    from jax.experimental.shard_map import shard_map
    from jax.sharding import Mesh, PartitionSpec

    from concourse.bass2jax import (
        _bass_exec_p,
        install_neuronx_cc_hook,
        partition_id_tensor,
    )

    install_neuronx_cc_hook()
    partition_name = nc.partition_id_tensor.name if nc.partition_id_tensor else None
    in_names, out_names, out_avals, zero_outs = [], [], [], []
    for alloc in nc.m.functions[0].allocations:
        if not isinstance(alloc, mybir.MemoryLocationSet):
            continue
        name = alloc.memorylocations[0].name
        if alloc.kind == "ExternalInput":
            if name != partition_name:
                in_names.append(name)
        elif alloc.kind == "ExternalOutput":
            out_names.append(name)
            shape = tuple(alloc.tensor_shape)
            dtype = mybir.dt.np(alloc.dtype)
            out_avals.append(jax.core.ShapedArray(shape, dtype))
            zero_outs.append(np.zeros(shape, dtype))
    n_params = len(in_names)
    all_in_names = list(in_names) + list(out_names)
    if partition_name is not None:
        all_in_names.append(partition_name)

    def _body(*args):
        operands = list(args)
        if partition_name is not None:
            operands.append(partition_id_tensor())
        outs = _bass_exec_p.bind(
            *operands,
            out_avals=tuple(out_avals),
            in_names=tuple(all_in_names),
            out_names=tuple(out_names),
            lowering_input_output_aliases=(),
            sim_require_finite=True,
            sim_require_nnan=True,
            nc=nc,
        )
        return tuple(outs)

    n_cores = nc.num_devices
    devices = jax.devices()[:n_cores]
    mesh = Mesh(np.asarray(devices), ("core",))
    in_specs = (PartitionSpec("core"),) * (n_params + len(out_names))
    out_specs = (PartitionSpec("core"),) * len(out_names)
    fn = jax.jit(
        shard_map(
            _body, mesh=mesh, in_specs=in_specs, out_specs=out_specs, check_rep=False
        ),
        keep_unused=True,
    )
    return fn, in_names, out_names, zero_outs


_RUNNER_CACHE = {}


def get_runner(nc, in_maps):
    """Return (fn, dev_args) for repeated dispatch of `nc` with `in_maps`."""
    import jax
    from jax.sharding import Mesh, NamedSharding, PartitionSpec

    key = id(nc)
    if key not in _RUNNER_CACHE:
        _RUNNER_CACHE[key] = _build_runner(nc)
    fn, in_names, out_names, zero_outs = _RUNNER_CACHE[key]
    n_cores = nc.num_devices
    mesh = Mesh(np.asarray(jax.devices()[:n_cores]), ("core",))
    shard = NamedSharding(mesh, PartitionSpec("core"))
    concat_in = [
        np.concatenate([np.asarray(in_maps[c][n]) for c in range(n_cores)], axis=0)
        for n in in_names
    ]
    dev_in = [jax.device_put(a, shard) for a in concat_in]
    zkey = ("zeros", key)
    if zkey not in _RUNNER_CACHE:
        concat_zeros = [
            np.zeros((n_cores * z.shape[0], *z.shape[1:]), z.dtype) for z in zero_outs
        ]
        _RUNNER_CACHE[zkey] = [jax.device_put(a, shard) for a in concat_zeros]
    return fn, dev_in + _RUNNER_CACHE[zkey]


def run_cached(nc, in_maps):
    """Execute via the cached runner; returns per-core result dicts."""
    fn, dev_args = get_runner(nc, in_maps)
    out_arrs = fn(*dev_args)
    _, _, out_names, zero_outs = _RUNNER_CACHE[id(nc)]
    n_cores = nc.num_devices
    fetched = [
        np.asarray(a).reshape(n_cores, *zero_outs[i].shape)
        for i, a in enumerate(out_arrs)
    ]
    return [
        {name: fetched[i][c] for i, name in enumerate(out_names)}
        for c in range(n_cores)
    ]


def make_in_maps(x, mask, w_qkv, w_proj):
    return [_prep_core_inputs(x, mask, w_qkv, w_proj, c) for c in range(N_CORES)]


def combine(results, b_proj):
    outs = []
    for b in range(B):
        outs.append(results[2 * b]["y"] + results[2 * b + 1]["y"] + b_proj[None, :])
    return np.stack(outs).astype(np.float32)


def kernel(x, mask, w_qkv, w_proj, b_proj):
    x = np.asarray(x, dtype=np.float32)
    mask = np.asarray(mask)
    w_qkv = np.asarray(w_qkv, dtype=np.float32)
    w_proj = np.asarray(w_proj, dtype=np.float32)
    b_proj = np.asarray(b_proj, dtype=np.float32)

    nc = get_nc()
    in_maps = make_in_maps(x, mask, w_qkv, w_proj)
    try:
        results = run_cached(nc, in_maps)
    except Exception:
        results = run_bass_kernel_spmd(nc, in_maps, list(range(N_CORES))).results
    return combine(results, b_proj)
